# revision 20
# baseline (speedup 1.0000x reference)
"""Trainium2 Bass kernel for nn_DeCoR (deformable 1-D conv, B=8 L=4096 C=256 K=3).

Data-parallel over batch: 1 batch element per NeuronCore, 8 cores.

Per core:
  1. Transpose x -> h [C, L] via PE transposes (matmuls contract over partitions).
  2. conv1 (C->C, k=3, SAME) + ReLU as PSUM-accumulated fp32r matmuls.
  3. conv2 (C->3, k=3, SAME) -> offsets off [3, L].
  4. PE-transpose offsets into l-on-partitions tiles on a window-aligned
     (p-4) domain; compute interp weights g0/g1 and shift d' = U0-l with an
     exact compare-based floor.
  5. interp + final conv factorizes as out[l] = sum_{k,j} w_j[l,k]*Y_k[l+j],
     Y_k = Wf_k^T h.  Per 124-column tile the j-sum is a 5-diagonal band
     matrix: stage 2 = matmul with banded stationary operand.  The band is
     materialized via a DRAM shear round-trip (write pitch 133, read pitch
     132; off-band stays zero), one batched write + grouped reads.
"""

import os
import sys
import threading

for _p in ("/opt/trn_rl_repo",):
    if _p not in sys.path and os.path.isdir(_p):
        sys.path.insert(0, _p)

import numpy as np

import concourse.bass as bass
import concourse.bacc as bacc
import concourse.tile as tile
import concourse.mybir as mybir
from concourse.bass_utils import run_bass_kernel_spmd

F32 = mybir.dt.float32
F32R = mybir.dt.float32r
U32 = mybir.dt.uint32
AF = mybir.ActivationFunctionType
OP = mybir.AluOpType

B, L, C, K = 8, 4096, 256, 3
N_CORES = 8
TS = 124           # stage-2 output columns per band tile
NT = 34            # band tiles (34*124 = 4216 >= 4096)
KT = K * NT        # 102 (k,t) columns, k-major
WIN = 128          # stage-2 window rows
HCOLS = 4224       # h cols per c-block: col = 1 + l
OCOLS = 4098       # relu(conv1) cols per block: col = 1 + l
FCOLS = 4224       # off cols: col = 4 + l
BW = 133           # band DRAM write pitch (elements)
BR = 132           # band DRAM read pitch
WTOT = 2 * (768 + 9 + 768)   # packed weight columns per c-block: 1545*2
GRP = 4            # band-load / y-store tile grouping


def build_kernel():
    nc = bacc.Bacc("TRN2", target_bir_lowering=False, debug=False,
                   num_devices=N_CORES)

    x_d = nc.dram_tensor("x", [L, C], F32, kind="ExternalInput")
    w_d = nc.dram_tensor("wpack", [2, 128, 1545], F32, kind="ExternalInput")
    geo_d = nc.dram_tensor("geo", [4, 128, KT], F32, kind="ExternalInput")
    id_d = nc.dram_tensor("ident", [128, 128], F32, kind="ExternalInput")
    y_d = nc.dram_tensor("y", [L, C], F32, kind="ExternalOutput")
    # band shear buffer: (k,t)-major blocks of 128 rows x 133 pitch
    bdram = nc.dram_tensor("bdram", [KT, 128, BW], F32R)

    with tile.TileContext(nc) as tc:
        with tc.tile_pool(name="perm", bufs=1) as perm:
            h_sb = perm.tile([128, 2 * HCOLS], F32R, tag="h")
            w_sb = perm.tile([128, 2 * 1545], F32R, tag="w")
            id_sb = perm.tile([128, 128], F32, tag="id")
            geo_sb = perm.tile([128, 4 * KT], F32, tag="geo")
            offt = perm.tile([128, KT], F32, tag="offt")
            tT = perm.tile([128, KT], F32, tag="tT")
            tc_t = perm.tile([128, KT], F32, tag="tc")
            tl_t = perm.tile([128, KT], F32, tag="tl")
            f1_t = perm.tile([128, KT], F32, tag="f1")
            f2_t = perm.tile([128, KT], F32, tag="f2")
            dl_t = perm.tile([128, KT], F32, tag="dl")
            s_t = perm.tile([128, KT], F32, tag="s")
            a_t = perm.tile([128, KT], F32, tag="a")
            gpack = perm.tile([128, 3 * KT], F32, tag="gpack")
            gs = perm.tile([128, 4 * 3 * KT], F32, tag="gs")
            zc = perm.tile([128, 5 * KT], F32, tag="zc")
            tt0 = perm.tile([128, KT], F32, tag="tt0")
            tt1 = perm.tile([128, KT], F32, tag="tt1")
            cp1 = perm.tile([128, 1], F32, tag="cp1")
            cm1 = perm.tile([128, 1], F32, tag="cm1")
            zsb = perm.tile([128, 8 * BW], F32R, tag="zsb")
            midcm = tc.tile_pool(name="mid", bufs=1)
            midp = midcm.__enter__()
            or_sb = midp.tile([128, 2 * OCOLS], F32R, tag="or")
            off_sb = midp.tile([3, FCOLS], F32, tag="off")

            base_c = geo_sb[0:128, 0 * KT:1 * KT]
            lcol_c = geo_sb[0:128, 1 * KT:2 * KT]
            lcol2_c = geo_sb[0:128, 2 * KT:3 * KT]
            lclip_c = geo_sb[0:128, 3 * KT:4 * KT]

            def wslice(cb, off, n):
                return w_sb[0:128, cb * 1545 + off:cb * 1545 + off + n]

            # ---- x prefetch (issue before consts so DMA starts on x) ----
            xts = []
            xcm = tc.tile_pool(name="xst", bufs=1)
            xpool = xcm.__enter__()
            for t2 in range(8):
                xt = xpool.tile([128, 1024], F32, tag=f"xs{t2}", bufs=1)
                srcx = x_d.ap()[512 * t2:512 * (t2 + 1), :] \
                    .rearrange("(i p) c -> p i c", p=128)
                nc.sync.dma_start(
                    xt[:].rearrange("p (i c) -> p i c", c=256), srcx)
                xts.append(xt)

            # ---- constant loads ----
            nc.sync.dma_start(w_sb[:],
                              w_d.ap().transpose([1, 0, 2]).bitcast(F32R))
            nc.sync.dma_start(geo_sb[:], geo_d.ap().transpose([1, 0, 2]))
            nc.sync.dma_start(id_sb[:], id_d.ap()[:])

            # ---- zero inits ----
            nc.vector.memset(zsb[:].bitcast(U32), 0)
            nc.vector.memset(gs[:], 0.0)
            for r in range(1, 5):
                nc.vector.memset(gs[0:128, (r - 1) * 3 * KT + 2 * KT:
                                 (r - 1) * 3 * KT + 3 * KT], 9.0)
            nc.vector.memset(cp1[:], 1.0)
            nc.vector.memset(cm1[:], -1.0)

            for cb in range(2):
                o = cb * HCOLS
                nc.vector.memset(h_sb[0:128, o:o + 1].bitcast(U32), 0)
                nc.vector.memset(h_sb[0:128, o + 1 + L:o + HCOLS].bitcast(U32), 0)
                o = cb * OCOLS
                nc.vector.memset(or_sb[0:128, o:o + 1].bitcast(U32), 0)
                nc.vector.memset(or_sb[0:128, o + 1 + L:o + OCOLS].bitcast(U32), 0)
            nc.vector.memset(off_sb[0:3, 0:4], 0.0)
            nc.vector.memset(off_sb[0:3, 4 + L:FCOLS], 0.0)

            # ---- phase A: transpose x -> h [c, l] ----
            with tc.tile_pool(name="psA", bufs=4, space="PSUM") as psa:
                for t2 in range(8):
                    xt = xts[t2]
                    for i in range(4):
                        t = 4 * t2 + i
                        for cb in range(2):
                            pt = psa.tile([128, 128], F32, tag="pt")
                            nc.tensor.transpose(
                                pt[:], xt[:, i * 256 + cb * 128:
                                          i * 256 + (cb + 1) * 128], id_sb[:])
                            dst = h_sb[0:128, cb * HCOLS + 1 + 128 * t:
                                       cb * HCOLS + 1 + 128 * (t + 1)]
                            if cb == 0:
                                nc.scalar.copy(dst, pt[:])
                            else:
                                nc.vector.tensor_copy(dst, pt[:])
            xcm.__exit__(None, None, None)

            # ---- phase B: conv1 + relu ----
            with tc.tile_pool(name="psB", bufs=3, space="PSUM") as psb:
                for lt in range(8):
                    l0 = 512 * lt
                    for mb in range(2):
                        pb = psb.tile([128, 512], F32, tag="pb")
                        first = True
                        for cb in range(2):
                            for k in range(K):
                                lhsT = wslice(cb, k * 256 + mb * 128, 128)
                                rhs = h_sb[0:128, cb * HCOLS + l0 + k:
                                           cb * HCOLS + l0 + k + 512]
                                nc.tensor.matmul(pb[:], lhsT, rhs, start=first,
                                                 stop=(cb == 1 and k == K - 1))
                                first = False
                        nc.scalar.activation(
                            or_sb[0:128, mb * OCOLS + 1 + l0:
                                  mb * OCOLS + 1 + l0 + 512], pb[:], AF.Relu)

            # ---- phase C: conv2 -> off ----
            with tc.tile_pool(name="psC", bufs=2, space="PSUM") as psc:
                for lt in range(8):
                    l0 = 512 * lt
                    pc = psc.tile([3, 512], F32, tag="pc")
                    first = True
                    for cb in range(2):
                        for k in range(K):
                            lhsT = wslice(cb, 768 + k * 3, 3)
                            rhs = or_sb[0:128, cb * OCOLS + l0 + k:
                                        cb * OCOLS + l0 + k + 512]
                            nc.tensor.matmul(pc[:], lhsT, rhs, start=first,
                                             stop=(cb == 1 and k == K - 1))
                            first = False
                    nc.vector.tensor_copy(off_sb[0:3, 4 + l0:4 + l0 + 512],
                                          pc[:])

            # ---- phase C2: window transposes off -> offt[p, k*34+t] ----
            # offt[p, (k,t)] = off[k, l] at l = 124 t + p - 4
            with tc.tile_pool(name="psC2", bufs=3, space="PSUM") as psc2:
                CG = 4
                for tg in range((NT + CG - 1) // CG):
                    t0c = tg * CG
                    gnc = min(CG, NT - t0c)
                    pt = psc2.tile([128, CG * 3], F32, tag="pt2")
                    for j in range(gnc):
                        nc.tensor.transpose(
                            pt[:, 3 * j:3 * j + 3],
                            off_sb[0:3, TS * (t0c + j):TS * (t0c + j) + 128],
                            id_sb[0:3, 0:3], )
                    # offt col = 3*t + k  (t-major)
                    dst = offt[0:128, 3 * t0c:3 * (t0c + gnc)]
                    srcp = pt[0:128, 0:3 * gnc]
                    if tg % 2 == 0:
                        nc.vector.tensor_copy(dst, srcp)
                    else:
                        nc.scalar.copy(dst, srcp)

            # ---- phase D: interp weights on the p-4 domain ----
            g0_c = gpack[0:128, 0:KT]
            g1_c = gpack[0:128, KT:2 * KT]
            dcl_c = gpack[0:128, 2 * KT:3 * KT]
            nc.vector.tensor_tensor(tT[:], offt[:], base_c, OP.add)
            nc.vector.tensor_tensor(tc_t[:], tT[:], lcol_c, OP.max)
            nc.vector.tensor_tensor(tc_t[:], tc_t[:], lcol2_c, OP.min)
            nc.vector.tensor_tensor(tl_t[:], tc_t[:], lcol_c, OP.subtract)
            nc.vector.tensor_scalar(f1_t[:], tl_t[:], 1.0, None, OP.is_ge)
            nc.vector.tensor_scalar(f2_t[:], tl_t[:], 2.0, None, OP.is_ge)
            nc.vector.tensor_tensor(dl_t[:], f1_t[:], f2_t[:], OP.add)
            nc.vector.tensor_tensor(dcl_c, dl_t[:], lclip_c, OP.min)
            nc.vector.tensor_tensor(s_t[:], tl_t[:], dcl_c, OP.subtract)
            nc.scalar.activation(g0_c, s_t[:], AF.Relu, bias=cp1[:], scale=-1.0)
            nc.scalar.activation(a_t[:], s_t[:], AF.Abs, bias=cm1[:], scale=1.0)
            nc.scalar.activation(g1_c, a_t[:], AF.Relu, bias=cp1[:], scale=-1.0)

            # ---- phase E: partition-shifted copies r = 1..4 ----
            for r in range(1, 5):
                o = (r - 1) * 3 * KT
                nc.sync.dma_start(gs[0:128 - r, o:o + 3 * KT],
                                  gpack[r:128, 0:3 * KT])
                # wrap rows: l crosses into tile t+1 (col +3 inside each sub-block)
                dst = gs[128 - r:128, o:o + 3 * KT] \
                    .rearrange("p (b c) -> p b c", c=KT)[:, :, 0:KT - 3]
                srcw = gpack[4:4 + r, 0:3 * KT] \
                    .rearrange("p (b c) -> p b c", c=KT)[:, :, 3:KT]
                nc.sync.dma_start(dst, srcw)

            # ---- phase F: band values zc[p, (k,t)*5 + r] ----
            for r in range(5):
                if r == 0:
                    d_src, a_src, b_src = dcl_c, g0_c, g1_c
                else:
                    o = (r - 1) * 3 * KT
                    a_src = gs[0:128, o:o + KT]
                    b_src = gs[0:128, o + KT:o + 2 * KT]
                    d_src = gs[0:128, o + 2 * KT:o + 3 * KT]
                nc.vector.scalar_tensor_tensor(tt0[:], d_src, float(3 - r),
                                               a_src, OP.is_equal, OP.mult)
                nc.vector.scalar_tensor_tensor(tt1[:], d_src, float(2 - r),
                                               b_src, OP.is_equal, OP.mult)
                zdst = zc[:, r::5]
                nc.vector.tensor_tensor(zdst, tt0[:], tt1[:], OP.add)

            midcm.__exit__(None, None, None)

            # ---- band write: one DMA, (p, kt, r) -> flat kt*17024+133p+r ----
            # zero-fill the band dram in chunks (stride-0 src is broken on HW DGE)
            zf_is = []
            ZG = 8
            for zg in range((KT + ZG - 1) // ZG):
                gn = min(ZG, KT - zg * ZG)
                zdst = bdram.ap()[zg * ZG:zg * ZG + gn].transpose([1, 0, 2])
                zsrc = zsb[0:128, 0:gn * BW].rearrange("p (k c) -> p k c", c=BW)
                zf_is.append(nc.sync.dma_start(zdst, zsrc))
            bsrc = zc[:].rearrange("p (kt r) -> p kt r", r=5)
            bw_dst = bdram.ap()[:, :, 0:5].transpose([1, 0, 2])
            bw_i = nc.sync.dma_start(bw_dst, bsrc.bitcast(F32R))
            for zf_i in zf_is:
                tile.add_dep_helper(bw_i.ins, zf_i.ins,
                                    reason="band write after dram zero-fill")

            # ---- phases G+H: stage-1 Y + stage-2 band matmuls (pipelined) ----
            ngrp = (NT + GRP - 1) // GRP
            PD = 2
            with tc.tile_pool(name="bsb", bufs=1) as bpool, \
                 tc.tile_pool(name="ysb", bufs=1) as ypool, \
                 tc.tile_pool(name="osb", bufs=2) as opool, \
                 tc.tile_pool(name="psY", bufs=6, space="PSUM") as psy, \
                 tc.tile_pool(name="psO", bufs=2, space="PSUM") as pso:

                def emit_group_front(g):
                    t0 = g * GRP
                    gn = min(GRP, NT - t0)
                    bt = []
                    for k in range(K):
                        b_sb = bpool.tile([128, GRP * BR], F32R,
                                          tag=f"b{g % (PD + 1)}_{k}")
                        src = bdram.ap()[3 * t0 + k:3 * (t0 + gn - 1) + k + 1:3] \
                            .rearrange("g p c -> g (p c)")[:, 0:BR * 128] \
                            .rearrange("g (p c) -> p g c", c=BR)
                        ld_i = nc.sync.dma_start(b_sb[0:128, 0:gn * BR], src)
                        tile.add_dep_helper(ld_i.ins, bw_i.ins,
                                            reason="band read after band write")
                        bt.append(b_sb)
                    ys = {}
                    for j in range(gn):
                        t = t0 + j
                        for k in range(K):
                            y_ps = psy.tile([128, 256], F32, tag="yps")
                            for cb in range(2):
                                lhsT = h_sb[0:128, cb * HCOLS + TS * t:
                                            cb * HCOLS + TS * t + WIN]
                                rhs = wslice(cb, 777 + k * 256, 256)
                                nc.tensor.matmul(y_ps[:], lhsT, rhs,
                                                 start=(cb == 0), stop=(cb == 1))
                            y_sb = ypool.tile([128, 256], F32R,
                                              tag=f"y{g % (PD + 1)}_{j}_{k}")
                            if (j * K + k) % 3 != 1:
                                nc.vector.tensor_copy(y_sb[:], y_ps[:])
                            else:
                                nc.scalar.copy(y_sb[:], y_ps[:])
                            ys[(j, k)] = y_sb
                    return bt, ys

                def emit_group_back(g, bt, ys):
                    t0 = g * GRP
                    gn = min(GRP, NT - t0)
                    o_g = opool.tile([TS, GRP * 256], F32, tag="og")
                    for j in range(gn):
                        o_ps = pso.tile([TS, 256], F32, tag="ops")
                        for k in range(K):
                            nc.tensor.matmul(
                                o_ps[:], bt[k][0:128, j * BR + 4:j * BR + 128],
                                ys[(j, k)][:], start=(k == 0), stop=(k == K - 1))
                        dst = o_g[0:TS, j * 256:(j + 1) * 256]
                        if j % 2 == 0:
                            nc.vector.tensor_copy(dst, o_ps[:])
                        else:
                            nc.scalar.copy(dst, o_ps[:])
                    full = [j for j in range(gn) if TS * (t0 + j) + TS <= L]
                    nf = len(full)
                    if nf:
                        ydst = y_d.ap()[TS * t0:TS * (t0 + nf), :] \
                            .rearrange("(g p) c -> p g c", p=TS)
                        ysrc = o_g[0:TS, 0:nf * 256] \
                            .rearrange("p (g c) -> p g c", c=256)
                        nc.sync.dma_start(ydst, ysrc)
                    if nf < gn:
                        t = t0 + nf
                        v = L - TS * t
                        nc.sync.dma_start(
                            y_d.ap()[TS * t:L, :],
                            o_g[0:v, nf * 256:nf * 256 + 256])

                state = {}
                for g in range(ngrp + PD):
                    if g < ngrp:
                        state[g] = emit_group_front(g)
                    if g >= PD:
                        emit_group_back(g - PD, *state.pop(g - PD))

    nc.compile()
    return nc


_LOCK = threading.Lock()
_CACHE = {}


def _get_nc():
    with _LOCK:
        if "nc" not in _CACHE:
            _CACHE["nc"] = build_kernel()
        return _CACHE["nc"]


def _host_inputs(w_conv1, w_conv2, w_final):
    wpack = np.empty((2, 128, 1545), np.float32)
    for cb in range(2):
        for k in range(K):
            wpack[cb, :, k * 256:(k + 1) * 256] = \
                w_conv1[:, cb * 128:(cb + 1) * 128, k].T
            wpack[cb, :, 768 + k * 3:768 + (k + 1) * 3] = \
                w_conv2[:, cb * 128:(cb + 1) * 128, k].T
            wpack[cb, :, 777 + k * 256:777 + (k + 1) * 256] = \
                w_final[:, cb * 128:(cb + 1) * 128, k].T
    p = np.arange(128, dtype=np.float64)[:, None]
    kt = np.arange(KT, dtype=np.float64)[None, :]
    tcol = kt // K
    kcol = kt % K
    lmat = TS * tcol + p - 4.0
    geo = np.empty((4, 128, KT), np.float32)
    geo[0] = lmat + kcol        # base = l + k
    geo[1] = lmat               # lcol
    geo[2] = lmat + 2.0         # lcol2
    geo[3] = (L - 2.0) - lmat   # lclip
    ident = np.eye(128, dtype=np.float32)
    return dict(wpack=wpack, geo=geo, ident=ident)


def kernel(x, w_conv1, w_conv2, w_final):
    x = np.ascontiguousarray(x, np.float32)
    consts = _host_inputs(np.asarray(w_conv1, np.float32),
                          np.asarray(w_conv2, np.float32),
                          np.asarray(w_final, np.float32))
    nc = _get_nc()
    in_maps = [dict(consts, x=np.ascontiguousarray(x[i])) for i in range(N_CORES)]
    res = run_bass_kernel_spmd(nc, in_maps, core_ids=list(range(N_CORES)))
    out = np.stack([res.results[i]["y"] for i in range(N_CORES)], axis=0)
    return out.astype(np.float32)


if __name__ == "__main__":
    rng = np.random.default_rng(0)
    xs = rng.standard_normal((B, L, C), dtype=np.float32)
    w1 = rng.standard_normal((C, C, K), dtype=np.float32) * 0.05
    w2 = rng.standard_normal((K, C, K), dtype=np.float32) * 0.05
    wf = rng.standard_normal((C, C, K), dtype=np.float32)
    y = kernel(xs, w1, w2, wf)
    print("out", y.shape, y.dtype, np.abs(y).mean())


# revision 21
# speedup vs baseline: 1.0859x; 1.0859x over previous
"""Trainium2 Bass kernel for nn_DeCoR (deformable 1-D conv, B=8 L=4096 C=256 K=3).

Data-parallel over batch: 1 batch element per NeuronCore, 8 cores.

Per core:
  1. Transpose x -> h [C, L] via PE transposes (matmuls contract over partitions).
  2. conv1 (C->C, k=3, SAME) + ReLU as PSUM-accumulated fp32r matmuls.
  3. conv2 (C->3, k=3, SAME) -> offsets off [3, L].
  4. PE-transpose offsets into l-on-partitions tiles on a window-aligned
     (p-4) domain; compute interp weights g0/g1 and shift d' = U0-l with an
     exact compare-based floor.
  5. interp + final conv factorizes as out[l] = sum_{k,j} w_j[l,k]*Y_k[l+j],
     Y_k = Wf_k^T h.  Per 124-column tile the j-sum is a 5-diagonal band
     matrix: stage 2 = matmul with banded stationary operand.  The band is
     materialized via a DRAM shear round-trip (write pitch 133, read pitch
     132; off-band stays zero), one batched write + grouped reads.
"""

import os
import sys
import threading

for _p in ("/opt/trn_rl_repo",):
    if _p not in sys.path and os.path.isdir(_p):
        sys.path.insert(0, _p)

import numpy as np

import concourse.bass as bass
import concourse.bacc as bacc
import concourse.tile as tile
import concourse.mybir as mybir
from concourse.bass_utils import run_bass_kernel_spmd

F32 = mybir.dt.float32
F32R = mybir.dt.float32r
BF16 = mybir.dt.bfloat16
U32 = mybir.dt.uint32
AF = mybir.ActivationFunctionType
OP = mybir.AluOpType

B, L, C, K = 8, 4096, 256, 3
N_CORES = 8
TS = 124           # stage-2 output columns per band tile
NT = 34            # band tiles (34*124 = 4216 >= 4096)
KT = K * NT        # 102 (k,t) columns, k-major
WIN = 128          # stage-2 window rows
HCOLS = 4224       # h cols per c-block: col = 1 + l
OCOLS = 4098       # relu(conv1) cols per block: col = 1 + l
FCOLS = 4224       # off cols: col = 4 + l
BW = 133           # band DRAM write pitch (elements)
BR = 132           # band DRAM read pitch
WTOT = 2 * (768 + 9 + 768)   # packed weight columns per c-block: 1545*2
GRP = 4            # band-load / y-store tile grouping


def build_kernel():
    nc = bacc.Bacc("TRN2", target_bir_lowering=False, debug=False,
                   num_devices=N_CORES)

    x_d = nc.dram_tensor("x", [L, C], F32, kind="ExternalInput")
    w_d = nc.dram_tensor("wpack", [2, 128, 1545], F32, kind="ExternalInput")
    geo_d = nc.dram_tensor("geo", [4, 128, KT], F32, kind="ExternalInput")
    id_d = nc.dram_tensor("ident", [128, 128], F32, kind="ExternalInput")
    y_d = nc.dram_tensor("y", [L, C], F32, kind="ExternalOutput")
    # band shear buffer: (k,t)-major blocks of 128 rows x 133 pitch
    bdram = nc.dram_tensor("bdram", [KT, 128, BW], BF16)

    with tile.TileContext(nc) as tc:
        with tc.tile_pool(name="perm", bufs=1) as perm:
            h_sb = perm.tile([128, 2 * HCOLS], F32R, tag="h")
            w_sb = perm.tile([128, 2 * 1545], F32R, tag="w")
            id_sb = perm.tile([128, 128], F32, tag="id")
            geo_sb = perm.tile([128, 4 * KT], F32, tag="geo")
            offt = perm.tile([128, KT], F32, tag="offt")
            tT = perm.tile([128, KT], F32, tag="tT")
            tc_t = perm.tile([128, KT], F32, tag="tc")
            tl_t = perm.tile([128, KT], F32, tag="tl")
            f1_t = perm.tile([128, KT], F32, tag="f1")
            f2_t = perm.tile([128, KT], F32, tag="f2")
            dl_t = perm.tile([128, KT], F32, tag="dl")
            s_t = perm.tile([128, KT], F32, tag="s")
            a_t = perm.tile([128, KT], F32, tag="a")
            gpack = perm.tile([128, 3 * KT], F32, tag="gpack")
            gs = perm.tile([128, 4 * 3 * KT], F32, tag="gs")
            zc = perm.tile([128, 5 * KT], BF16, tag="zc")
            tt0 = perm.tile([128, KT], F32, tag="tt0")
            tt1 = perm.tile([128, KT], F32, tag="tt1")
            cp1 = perm.tile([128, 1], F32, tag="cp1")
            cm1 = perm.tile([128, 1], F32, tag="cm1")
            zsb = perm.tile([128, 8 * BW], BF16, tag="zsb")
            midcm = tc.tile_pool(name="mid", bufs=1)
            midp = midcm.__enter__()
            or_sb = midp.tile([128, 2 * OCOLS], F32R, tag="or")
            off_sb = midp.tile([3, FCOLS], F32, tag="off")

            base_c = geo_sb[0:128, 0 * KT:1 * KT]
            lcol_c = geo_sb[0:128, 1 * KT:2 * KT]
            lcol2_c = geo_sb[0:128, 2 * KT:3 * KT]
            lclip_c = geo_sb[0:128, 3 * KT:4 * KT]

            def wslice(cb, off, n):
                return w_sb[0:128, cb * 1545 + off:cb * 1545 + off + n]

            # ---- head DMAs: x chunk 0, ident (gates transposes), w, rest of x
            xts = []
            xcm = tc.tile_pool(name="xst", bufs=1)
            xpool = xcm.__enter__()

            def load_x(t2):
                xt = xpool.tile([128, 1024], F32, tag=f"xs{t2}", bufs=1)
                srcx = x_d.ap()[512 * t2:512 * (t2 + 1), :] \
                    .rearrange("(i p) c -> p i c", p=128)
                nc.sync.dma_start(
                    xt[:].rearrange("p (i c) -> p i c", c=256), srcx)
                xts.append(xt)

            load_x(0)
            nc.sync.dma_start(id_sb[:], id_d.ap()[:])
            load_x(1)
            nc.sync.dma_start(w_sb[:],
                              w_d.ap().transpose([1, 0, 2]).bitcast(F32R))
            for _t2 in range(2, 8):
                load_x(_t2)
            nc.sync.dma_start(geo_sb[:], geo_d.ap().transpose([1, 0, 2]))

            # ---- zero inits ----
            nc.vector.memset(zsb[:], 0.0)
            nc.vector.memset(gs[:], 0.0)
            for r in range(1, 5):
                nc.vector.memset(gs[0:128, (r - 1) * 3 * KT + 2 * KT:
                                 (r - 1) * 3 * KT + 3 * KT], 9.0)
            nc.vector.memset(cp1[:], 1.0)
            nc.vector.memset(cm1[:], -1.0)

            for cb in range(2):
                o = cb * HCOLS
                nc.vector.memset(h_sb[0:128, o:o + 1].bitcast(U32), 0)
                nc.vector.memset(h_sb[0:128, o + 1 + L:o + HCOLS].bitcast(U32), 0)
                o = cb * OCOLS
                nc.vector.memset(or_sb[0:128, o:o + 1].bitcast(U32), 0)
                nc.vector.memset(or_sb[0:128, o + 1 + L:o + OCOLS].bitcast(U32), 0)
            nc.vector.memset(off_sb[0:3, 0:4], 0.0)
            nc.vector.memset(off_sb[0:3, 4 + L:FCOLS], 0.0)

            # ---- phase A: transpose x -> h [c, l] ----
            with tc.tile_pool(name="psA", bufs=4, space="PSUM") as psa:
                for t2 in range(8):
                    xt = xts[t2]
                    for i in range(4):
                        t = 4 * t2 + i
                        for cb in range(2):
                            pt = psa.tile([128, 128], F32, tag="pt")
                            nc.tensor.transpose(
                                pt[:], xt[:, i * 256 + cb * 128:
                                          i * 256 + (cb + 1) * 128], id_sb[:])
                            dst = h_sb[0:128, cb * HCOLS + 1 + 128 * t:
                                       cb * HCOLS + 1 + 128 * (t + 1)]
                            if cb == 0:
                                nc.scalar.copy(dst, pt[:])
                            else:
                                nc.vector.tensor_copy(dst, pt[:])
            xcm.__exit__(None, None, None)

            # ---- phase B: conv1 + relu ----
            with tc.tile_pool(name="psB", bufs=3, space="PSUM") as psb:
                for lt in range(8):
                    l0 = 512 * lt
                    for mb in range(2):
                        pb = psb.tile([128, 512], F32, tag="pb")
                        first = True
                        for cb in range(2):
                            for k in range(K):
                                lhsT = wslice(cb, k * 256 + mb * 128, 128)
                                rhs = h_sb[0:128, cb * HCOLS + l0 + k:
                                           cb * HCOLS + l0 + k + 512]
                                nc.tensor.matmul(pb[:], lhsT, rhs, start=first,
                                                 stop=(cb == 1 and k == K - 1))
                                first = False
                        nc.scalar.activation(
                            or_sb[0:128, mb * OCOLS + 1 + l0:
                                  mb * OCOLS + 1 + l0 + 512], pb[:], AF.Relu)

            # ---- phase C: conv2 -> off ----
            with tc.tile_pool(name="psC", bufs=2, space="PSUM") as psc:
                for lt in range(8):
                    l0 = 512 * lt
                    pc = psc.tile([3, 512], F32, tag="pc")
                    first = True
                    for cb in range(2):
                        for k in range(K):
                            lhsT = wslice(cb, 768 + k * 3, 3)
                            rhs = or_sb[0:128, cb * OCOLS + l0 + k:
                                        cb * OCOLS + l0 + k + 512]
                            nc.tensor.matmul(pc[:], lhsT, rhs, start=first,
                                             stop=(cb == 1 and k == K - 1))
                            first = False
                    nc.vector.tensor_copy(off_sb[0:3, 4 + l0:4 + l0 + 512],
                                          pc[:])

            # ---- phase C2: window transposes off -> offt[p, k*34+t] ----
            # offt[p, (k,t)] = off[k, l] at l = 124 t + p - 4
            with tc.tile_pool(name="psC2", bufs=3, space="PSUM") as psc2:
                CG = 4
                for tg in range((NT + CG - 1) // CG):
                    t0c = tg * CG
                    gnc = min(CG, NT - t0c)
                    pt = psc2.tile([128, CG * 3], F32, tag="pt2")
                    for j in range(gnc):
                        nc.tensor.transpose(
                            pt[:, 3 * j:3 * j + 3],
                            off_sb[0:3, TS * (t0c + j):TS * (t0c + j) + 128],
                            id_sb[0:3, 0:3], )
                    # offt col = 3*t + k  (t-major)
                    dst = offt[0:128, 3 * t0c:3 * (t0c + gnc)]
                    srcp = pt[0:128, 0:3 * gnc]
                    if tg % 2 == 0:
                        nc.vector.tensor_copy(dst, srcp)
                    else:
                        nc.scalar.copy(dst, srcp)

            # ---- phase D: interp weights on the p-4 domain ----
            g0_c = gpack[0:128, 0:KT]
            g1_c = gpack[0:128, KT:2 * KT]
            dcl_c = gpack[0:128, 2 * KT:3 * KT]
            nc.vector.tensor_tensor(tT[:], offt[:], base_c, OP.add)
            nc.vector.tensor_tensor(tc_t[:], tT[:], lcol_c, OP.max)
            nc.vector.tensor_tensor(tc_t[:], tc_t[:], lcol2_c, OP.min)
            nc.vector.tensor_tensor(tl_t[:], tc_t[:], lcol_c, OP.subtract)
            nc.vector.tensor_scalar(f1_t[:], tl_t[:], 1.0, None, OP.is_ge)
            nc.vector.tensor_scalar(f2_t[:], tl_t[:], 2.0, None, OP.is_ge)
            nc.vector.tensor_tensor(dl_t[:], f1_t[:], f2_t[:], OP.add)
            nc.vector.tensor_tensor(dcl_c, dl_t[:], lclip_c, OP.min)
            nc.vector.tensor_tensor(s_t[:], tl_t[:], dcl_c, OP.subtract)
            nc.scalar.activation(g0_c, s_t[:], AF.Relu, bias=cp1[:], scale=-1.0)
            nc.scalar.activation(a_t[:], s_t[:], AF.Abs, bias=cm1[:], scale=1.0)
            nc.scalar.activation(g1_c, a_t[:], AF.Relu, bias=cp1[:], scale=-1.0)

            # ---- phase E: partition-shifted copies r = 1..4 ----
            for r in range(1, 5):
                o = (r - 1) * 3 * KT
                nc.sync.dma_start(gs[0:128 - r, o:o + 3 * KT],
                                  gpack[r:128, 0:3 * KT])
                # wrap rows: l crosses into tile t+1 (col +3 inside each sub-block)
                dst = gs[128 - r:128, o:o + 3 * KT] \
                    .rearrange("p (b c) -> p b c", c=KT)[:, :, 0:KT - 3]
                srcw = gpack[4:4 + r, 0:3 * KT] \
                    .rearrange("p (b c) -> p b c", c=KT)[:, :, 3:KT]
                nc.sync.dma_start(dst, srcw)

            # ---- phase F: band values zc[p, (k,t)*5 + r] ----
            for r in range(5):
                if r == 0:
                    d_src, a_src, b_src = dcl_c, g0_c, g1_c
                else:
                    o = (r - 1) * 3 * KT
                    a_src = gs[0:128, o:o + KT]
                    b_src = gs[0:128, o + KT:o + 2 * KT]
                    d_src = gs[0:128, o + 2 * KT:o + 3 * KT]
                nc.vector.scalar_tensor_tensor(tt0[:], d_src, float(3 - r),
                                               a_src, OP.is_equal, OP.mult)
                nc.vector.scalar_tensor_tensor(tt1[:], d_src, float(2 - r),
                                               b_src, OP.is_equal, OP.mult)
                zdst = zc[:, r::5]
                nc.vector.tensor_tensor(zdst, tt0[:], tt1[:], OP.add)

            midcm.__exit__(None, None, None)

            # ---- band write: one DMA, (p, kt, r) -> flat kt*17024+133p+r ----
            # zero-fill the band dram in chunks (stride-0 src is broken on HW DGE)
            zf_is = []
            ZG = 8
            for zg in range((KT + ZG - 1) // ZG):
                gn = min(ZG, KT - zg * ZG)
                zdst = bdram.ap()[zg * ZG:zg * ZG + gn].transpose([1, 0, 2])
                zsrc = zsb[0:128, 0:gn * BW].rearrange("p (k c) -> p k c", c=BW)
                zf_is.append(nc.sync.dma_start(zdst, zsrc))
            bsrc = zc[:].rearrange("p (kt r) -> p kt r", r=5)
            bw_dst = bdram.ap()[:, :, 0:5].transpose([1, 0, 2])
            bw_i = nc.sync.dma_start(bw_dst, bsrc)
            for zf_i in zf_is:
                tile.add_dep_helper(bw_i.ins, zf_i.ins,
                                    reason="band write after dram zero-fill")

            # ---- phases G+H: stage-1 Y + stage-2 band matmuls (pipelined) ----
            ngrp = (NT + GRP - 1) // GRP
            PD = 2
            with tc.tile_pool(name="bsb", bufs=1) as bpool, \
                 tc.tile_pool(name="ysb", bufs=1) as ypool, \
                 tc.tile_pool(name="osb", bufs=2) as opool, \
                 tc.tile_pool(name="psY", bufs=6, space="PSUM") as psy, \
                 tc.tile_pool(name="psO", bufs=2, space="PSUM") as pso:

                def emit_group_front(g):
                    t0 = g * GRP
                    gn = min(GRP, NT - t0)
                    bt = []
                    for k in range(K):
                        b_sb = bpool.tile([128, GRP * BR], BF16,
                                          tag=f"b{g % (PD + 1)}_{k}")
                        src = bdram.ap()[3 * t0 + k:3 * (t0 + gn - 1) + k + 1:3] \
                            .rearrange("g p c -> g (p c)")[:, 0:BR * 128] \
                            .rearrange("g (p c) -> p g c", c=BR)
                        ld_i = nc.sync.dma_start(b_sb[0:128, 0:gn * BR], src)
                        tile.add_dep_helper(ld_i.ins, bw_i.ins,
                                            reason="band read after band write")
                        bt.append(b_sb)
                    ys = {}
                    for j in range(gn):
                        t = t0 + j
                        for k in range(K):
                            y_ps = psy.tile([128, 256], F32, tag="yps")
                            for cb in range(2):
                                lhsT = h_sb[0:128, cb * HCOLS + TS * t:
                                            cb * HCOLS + TS * t + WIN]
                                rhs = wslice(cb, 777 + k * 256, 256)
                                nc.tensor.matmul(y_ps[:], lhsT, rhs,
                                                 start=(cb == 0), stop=(cb == 1))
                            y_sb = ypool.tile([128, 256], BF16,
                                              tag=f"y{g % (PD + 1)}_{j}_{k}")
                            if (j * K + k) % 2 == 0:
                                nc.vector.tensor_copy(y_sb[:], y_ps[:])
                            else:
                                nc.scalar.copy(y_sb[:], y_ps[:])
                            ys[(j, k)] = y_sb
                    return bt, ys

                def emit_group_back(g, bt, ys):
                    t0 = g * GRP
                    gn = min(GRP, NT - t0)
                    o_g = opool.tile([TS, GRP * 256], F32, tag="og")
                    for j in range(gn):
                        o_ps = pso.tile([TS, 256], F32, tag="ops")
                        for k in range(K):
                            nc.tensor.matmul(
                                o_ps[:], bt[k][0:128, j * BR + 4:j * BR + 128],
                                ys[(j, k)][:], start=(k == 0), stop=(k == K - 1))
                        dst = o_g[0:TS, j * 256:(j + 1) * 256]
                        if j % 2 == 0:
                            nc.vector.tensor_copy(dst, o_ps[:])
                        else:
                            nc.scalar.copy(dst, o_ps[:])
                    full = [j for j in range(gn) if TS * (t0 + j) + TS <= L]
                    nf = len(full)
                    if nf:
                        ydst = y_d.ap()[TS * t0:TS * (t0 + nf), :] \
                            .rearrange("(g p) c -> p g c", p=TS)
                        ysrc = o_g[0:TS, 0:nf * 256] \
                            .rearrange("p (g c) -> p g c", c=256)
                        nc.sync.dma_start(ydst, ysrc)
                    if nf < gn:
                        t = t0 + nf
                        v = L - TS * t
                        nc.sync.dma_start(
                            y_d.ap()[TS * t:L, :],
                            o_g[0:v, nf * 256:nf * 256 + 256])

                state = {}
                for g in range(ngrp + PD):
                    if g < ngrp:
                        state[g] = emit_group_front(g)
                    if g >= PD:
                        emit_group_back(g - PD, *state.pop(g - PD))

    nc.compile()
    return nc


_LOCK = threading.Lock()
_CACHE = {}


def _get_nc():
    with _LOCK:
        if "nc" not in _CACHE:
            _CACHE["nc"] = build_kernel()
        return _CACHE["nc"]


def _host_inputs(w_conv1, w_conv2, w_final):
    wpack = np.empty((2, 128, 1545), np.float32)
    for cb in range(2):
        for k in range(K):
            wpack[cb, :, k * 256:(k + 1) * 256] = \
                w_conv1[:, cb * 128:(cb + 1) * 128, k].T
            wpack[cb, :, 768 + k * 3:768 + (k + 1) * 3] = \
                w_conv2[:, cb * 128:(cb + 1) * 128, k].T
            wpack[cb, :, 777 + k * 256:777 + (k + 1) * 256] = \
                w_final[:, cb * 128:(cb + 1) * 128, k].T
    p = np.arange(128, dtype=np.float64)[:, None]
    kt = np.arange(KT, dtype=np.float64)[None, :]
    tcol = kt // K
    kcol = kt % K
    lmat = TS * tcol + p - 4.0
    geo = np.empty((4, 128, KT), np.float32)
    geo[0] = lmat + kcol        # base = l + k
    geo[1] = lmat               # lcol
    geo[2] = lmat + 2.0         # lcol2
    geo[3] = (L - 2.0) - lmat   # lclip
    ident = np.eye(128, dtype=np.float32)
    return dict(wpack=wpack, geo=geo, ident=ident)


def kernel(x, w_conv1, w_conv2, w_final):
    x = np.ascontiguousarray(x, np.float32)
    consts = _host_inputs(np.asarray(w_conv1, np.float32),
                          np.asarray(w_conv2, np.float32),
                          np.asarray(w_final, np.float32))
    nc = _get_nc()
    in_maps = [dict(consts, x=np.ascontiguousarray(x[i])) for i in range(N_CORES)]
    res = run_bass_kernel_spmd(nc, in_maps, core_ids=list(range(N_CORES)))
    out = np.stack([res.results[i]["y"] for i in range(N_CORES)], axis=0)
    return out.astype(np.float32)


if __name__ == "__main__":
    rng = np.random.default_rng(0)
    xs = rng.standard_normal((B, L, C), dtype=np.float32)
    w1 = rng.standard_normal((C, C, K), dtype=np.float32) * 0.05
    w2 = rng.standard_normal((K, C, K), dtype=np.float32) * 0.05
    wf = rng.standard_normal((C, C, K), dtype=np.float32)
    y = kernel(xs, w1, w2, wf)
    print("out", y.shape, y.dtype, np.abs(y).mean())
